# revision 8
# baseline (speedup 1.0000x reference)
"""Trainium2 Bass kernel for nn_CrossLayerAggregation.

Strategy (data-parallel over batch, one batch element per NeuronCore):

Math refactoring (validated against the reference to ~1e-7 in fp32):
  * psi2(psi1(x)) is folded into a single stride-2 conv3 with weights
    W2f[:,:,k] = psi2_w[:,:,k] @ psi1_w  (biases are zero in this problem).
  * nearest-upsample x2 duplicates columns, so every up2down quantity
    lives at LUP resolution:  up2down[:, 2j] = up2down[:, 2j+1] = P[:, j]
    with P = (I + phi_w) @ U.
  * The [C,C] attention logits contract over L, and the tiny projection
    matrices commute out of the big matmuls:
        logits_d2u * sqrt(C) = Wq @ (U @ D^T) @ Wk^T
        logits_u2d * sqrt(C) = Wq @ (Dsum @ U^T) @ (I+phi)^T @ Wk^T
    where D = maxpool3s2(Dd) + conv3f(Dd) and Dsum[:,j] = Dd[:,2j]+Dd[:,2j+1].
  * Outputs:  up_out   = U  + (A1 @ Wv) @ D
              down_out = Dd + dup((A2 @ Wv @ (I+phi)) @ U)

Device pipeline per core:
  pass 1: stream U and Dd chunks; build D (PE conv + DVE maxpool) and Dsum;
          cast to bf16; xbar-DMA-transpose chunks to [l, c] layout; accumulate
          G1^T = D U^T-style contractions in PSUM.  D (bf16) is staged to a
          DRAM scratch for pass 2.
  small stage: [256,256] logits chains, softmax, attention matrices (PE + DVE
          + ACT, all tiny).
  pass 2: stream U, Dd, D16 chunks; two small-K matmuls per chunk; residual
          adds; write outputs.
"""

import os
import sys
import math

sys.path.insert(0, "/opt/trn_rl_repo")

import numpy as np

import concourse.bass as bass
import concourse.tile as tile
from concourse import bacc, mybir
from concourse.masks import make_identity

F32 = mybir.dt.float32
BF16 = mybir.dt.float16  # 16-bit staging dtype (fp16: more mantissa than bf16, data is O(10))
AF = mybir.ActivationFunctionType

C = 256
P = 128
SC = math.sqrt(C)


def _pack_lhsT(w):
    """[256,256] -> [128, 2, 2, 128]; tile (kk, m) = w[kk*128:(kk+1)*128, m*128:(m+1)*128].

    smm() with this packing computes  out = w.T @ rhs.
    """
    return np.ascontiguousarray(
        w.reshape(2, P, 2, P).transpose(1, 0, 2, 3)).astype(np.float32)


def build_program(LUP, LDOWN, n1, n2, num_devices=8):
    """Build the Bass program. Returns (nc, input_names)."""
    assert LDOWN == 2 * LUP
    nch1 = LUP // n1
    nch2 = LUP // n2
    nblk = n1 // P
    nsub = n1 // 512
    assert n1 % 512 == 0 and n2 == 512

    nc = bacc.Bacc("TRN2", target_bir_lowering=False, debug=False,
                   num_devices=num_devices)

    up_d = nc.dram_tensor("up", [C, LUP], F32, kind="ExternalInput")
    dn_d = nc.dram_tensor("down", [C, LDOWN], F32, kind="ExternalInput")
    w2f_d = nc.dram_tensor("w2f", [P, 3, 2, 2, P], F32, kind="ExternalInput")
    wkT_d = nc.dram_tensor("wkT", [P, 2, 2, P], F32, kind="ExternalInput")
    wqT_d = nc.dram_tensor("wqT", [P, 2, 2, P], F32, kind="ExternalInput")
    wphiT_d = nc.dram_tensor("wphiT", [P, 2, 2, P], F32, kind="ExternalInput")
    wphi_d = nc.dram_tensor("wphi", [P, 2, 2, P], F32, kind="ExternalInput")
    wv_d = nc.dram_tensor("wv", [P, 2, 2, P], F32, kind="ExternalInput")
    upo_d = nc.dram_tensor("up_out", [C, LUP], F32, kind="ExternalOutput")
    dno_d = nc.dram_tensor("down_out", [C, LDOWN], F32, kind="ExternalOutput")

    up_r = up_d.ap().rearrange("(m p) l -> p m l", p=P)
    dn_r = dn_d.ap().rearrange("(m p) l -> p m l", p=P)
    upo_r = upo_d.ap().rearrange("(m p) l -> p m l", p=P)
    dno_r = dno_d.ap().rearrange("(m p) l -> p m l", p=P)

    with tile.TileContext(nc) as tc:
        with (
            tc.tile_pool(name="singles", bufs=1) as singles,
            tc.tile_pool(name="small", bufs=1) as small,
            tc.tile_pool(name="dram", bufs=1, space="DRAM") as dram,
            tc.tile_pool(name="p1_in", bufs=2) as p1_in,
            tc.tile_pool(name="p1_work", bufs=2) as p1_work,
            tc.tile_pool(name="p1_tr", bufs=2) as p1_tr,
            tc.tile_pool(name="work_psum", bufs=2, space="PSUM") as work_psum,
            tc.tile_pool(name="acc_pool", bufs=1, space="PSUM") as acc_pool,
            tc.tile_pool(name="sm_psum", bufs=1, space="PSUM") as sm_psum,
            tc.tile_pool(name="p2_in", bufs=3) as p2_in,
            tc.tile_pool(name="p2_out", bufs=3) as p2_out,
        ):
            ident = singles.tile([P, P], F32, name="ident")
            make_identity(nc, ident)

            w2f_sb = singles.tile([P, 3, 2, 2, P], F32, name="w2f_sb")
            nc.sync.dma_start(w2f_sb, w2f_d.ap())
            wkT_sb = singles.tile([P, 2, 2, P], F32, name="wkT_sb")
            nc.sync.dma_start(wkT_sb, wkT_d.ap())
            wqT_sb = singles.tile([P, 2, 2, P], F32, name="wqT_sb")
            nc.sync.dma_start(wqT_sb, wqT_d.ap())
            wphiT_sb = singles.tile([P, 2, 2, P], F32, name="wphiT_sb")
            nc.sync.dma_start(wphiT_sb, wphiT_d.ap())
            wphi_sb = singles.tile([P, 2, 2, P], F32, name="wphi_sb")
            nc.sync.dma_start(wphi_sb, wphi_d.ap())
            wv_sb = singles.tile([P, 2, 2, P], F32, name="wv_sb")
            nc.sync.dma_start(wv_sb, wv_d.ap())

            d16_dram = dram.tile([P, 2, LUP], BF16, name="d16_dram")

            # ---------------- pass 1 ----------------
            # one PSUM bank per accumulator: start=True clears has_written at
            # bank granularity, so independent accumulation groups must not
            # share a bank.
            g1_ps = [acc_pool.tile([P, C], F32, name=f"g1_ps{m}")
                     for m in range(2)]
            ht_ps = [acc_pool.tile([P, C], F32, name=f"ht_ps{m}")
                     for m in range(2)]

            for i in range(nch1):
                c0 = i * n1
                u_f = p1_in.tile([P, 2, n1], F32, name="u_f")
                nc.sync.dma_start(u_f, up_r[:, :, c0:c0 + n1])
                # 2*n1 data cols + 1 left-halo col + 1 trailing pad col that is
                # never read (keeps the k=2 conv-tap slice in bounds).
                dn_f = p1_in.tile([P, 2, 2 * n1 + 2], F32, name="dn_f")
                if i == 0:
                    nc.vector.memset(dn_f[:, :, 0:1], 0.0)
                    nc.sync.dma_start(dn_f[:, :, 1:2 * n1 + 1],
                                      dn_r[:, :, 0:2 * n1])
                else:
                    nc.sync.dma_start(dn_f[:, :, 0:2 * n1 + 1],
                                      dn_r[:, :, 2 * c0 - 1:2 * c0 + 2 * n1])

                # maxpool3 stride2: out[t] = max(x[2t-1], x[2t], x[2t+1])
                dnp = dn_f[:, :, 1:1 + 2 * n1].rearrange(
                    "p m (n two) -> p m n two", two=2)
                sA = dnp[:, :, :, 0]
                sB = dnp[:, :, :, 1]
                sCm = dn_f[:, :, 0:2 * n1].rearrange(
                    "p m (n two) -> p m n two", two=2)[:, :, :, 0]
                t1 = p1_work.tile([P, 2, n1], F32, name="t1")
                nc.vector.tensor_max(t1, sA, sB)
                nc.vector.tensor_max(t1, t1, sCm)
                if i == 0:
                    # column 0: window is [pad, x0, x1] with pad = -inf, but the
                    # shared halo column holds conv's 0.0 — recompute it.
                    nc.vector.tensor_max(t1[:, :, 0:1], dn_f[:, :, 1:2],
                                         dn_f[:, :, 2:3])

                # folded conv3 stride-2 (PE), then D16 = maxpool + conv (bf16)
                d16 = p1_work.tile([P, 2, n1], BF16, name="d16")
                for m in range(2):
                    for ns in range(nsub):
                        psc = work_psum.tile([P, 512], F32, name="psc", tag="wps")
                        for k in range(3):
                            for kk in range(2):
                                rhs = dn_f[:, kk,
                                           k + ns * 1024:k + ns * 1024 + 1024
                                           ].rearrange("p (n two) -> p n two",
                                                       two=2)[:, :, 0]
                                nc.tensor.matmul(
                                    psc, w2f_sb[:, k, kk, m, :], rhs,
                                    start=(k == 0 and kk == 0),
                                    stop=(k == 2 and kk == 1))
                        sl = slice(ns * 512, (ns + 1) * 512)
                        nc.vector.tensor_add(d16[:, m, sl], t1[:, m, sl], psc)
                nc.sync.dma_start(d16_dram[:, :, c0:c0 + n1], d16)

                # Dsum and U in bf16
                s16 = p1_work.tile([P, 2, n1], BF16, name="s16")
                nc.vector.tensor_add(s16, sA, sB)
                u16 = p1_work.tile([P, 2, n1], BF16, name="u16")
                nc.scalar.activation(u16, u_f, AF.Copy)

                # xbar transposes to [l, c] layout
                ut = p1_tr.tile([P, nblk, C], BF16, name="ut")
                dt_ = p1_tr.tile([P, nblk, C], BF16, name="dt_")
                st = p1_tr.tile([P, nblk, C], BF16, name="st")
                for m in range(2):
                    ms = slice(m * P, (m + 1) * P)
                    nc.sync.dma_start(ut[:, :, ms], u16[:, m, :], transpose=True)
                    nc.sync.dma_start(dt_[:, :, ms], d16[:, m, :], transpose=True)
                    nc.sync.dma_start(st[:, :, ms], s16[:, m, :], transpose=True)

                # contractions: G1^T[d,c] += D[d,l] U[c,l];  H^T[d,c] += U[d,l] S[c,l]
                for b in range(nblk):
                    st_mm = (i == 0 and b == 0)
                    sp_mm = (i == nch1 - 1 and b == nblk - 1)
                    for m in range(2):
                        ms = slice(m * P, (m + 1) * P)
                        nc.tensor.matmul(g1_ps[m], dt_[:, b, ms],
                                         ut[:, b, :], start=st_mm, stop=sp_mm,
                                         skip_group_check=True)
                        nc.tensor.matmul(ht_ps[m], ut[:, b, ms],
                                         st[:, b, :], start=st_mm, stop=sp_mm,
                                         skip_group_check=True)

            # ---------------- small stage ----------------
            def psum_to_sbuf(name, ps_tiles):
                out = small.tile([P, 2, C], F32, name=name)
                for m in range(2):
                    nc.scalar.activation(out[:, m, :], ps_tiles[m], AF.Copy)
                return out

            g1_sb = psum_to_sbuf("g1_sb", g1_ps)   # = G1^T  [d, c]
            ht_sb = psum_to_sbuf("ht_sb", ht_ps)   # = H^T   [d, c]

            def smm(name, w_sb, rhs, out_dtype=F32):
                """out = W.T @ rhs for packed w_sb; rhs/out are [P, 2, C]."""
                out = small.tile([P, 2, C], out_dtype, name=name)
                for m in range(2):
                    ps = sm_psum.tile([P, C], F32, name=f"{name}_ps", tag="smm_ps")
                    for kk in range(2):
                        nc.tensor.matmul(ps, w_sb[:, kk, m, :], rhs[:, kk, :],
                                         start=(kk == 0), stop=(kk == 1))
                    nc.scalar.activation(out[:, m, :], ps, AF.Copy)
                return out

            def transpose4(name, src):
                out = small.tile([P, 2, C], F32, name=name)
                for mi in range(2):
                    for mj in range(2):
                        ps = sm_psum.tile([P, P], F32, name=f"{name}_tp",
                                          tag="tp_ps")
                        nc.tensor.transpose(
                            ps, src[:, mi, mj * P:(mj + 1) * P], ident)
                        nc.scalar.activation(
                            out[:, mj, mi * P:(mi + 1) * P], ps, AF.Copy)
                return out

            def softmax(name, src):
                out = small.tile([P, 2, C], F32, name=name)
                for m in range(2):
                    mx = small.tile([P, 1], F32, name=f"{name}_mx{m}")
                    nc.vector.reduce_max(out=mx, in_=src[:, m, :],
                                         axis=mybir.AxisListType.X)
                    nm = small.tile([P, 1], F32, name=f"{name}_nm{m}")
                    nc.vector.tensor_scalar_mul(nm, mx, -1.0 / SC)
                    e = out[:, m, :]
                    nc.scalar.activation(e, src[:, m, :], AF.Exp,
                                         bias=nm, scale=1.0 / SC)
                    s = small.tile([P, 1], F32, name=f"{name}_s{m}")
                    nc.vector.reduce_sum(out=s, in_=e, axis=mybir.AxisListType.X)
                    r = small.tile([P, 1], F32, name=f"{name}_r{m}")
                    nc.vector.reciprocal(r, s)
                    nc.vector.tensor_scalar_mul(e, e, r)
                return out

            # up path attention
            y1 = smm("y1", wkT_sb, g1_sb)            # Wk @ G1^T
            y1t = transpose4("y1t", y1)              # G1 @ Wk^T
            l1 = smm("l1", wqT_sb, y1t)              # logits1 * SC  [c, d]
            a1 = softmax("a1", l1)
            a1t = transpose4("a1t", a1)
            m1t = smm("m1t", wv_sb, a1t)             # M1^T = Wv^T @ A1^T
            m1t16 = small.tile([P, 2, C], BF16, name="m1t16")
            for m in range(2):
                nc.scalar.activation(m1t16[:, m, :], m1t[:, m, :], AF.Copy)

            # down path attention
            ddu2dt = smm("ddu2dt", wphiT_sb, ht_sb)  # (Dd @ U2D^T)^T
            y2 = smm("y2", wkT_sb, ddu2dt)
            y2t = transpose4("y2t", y2)
            l2 = smm("l2", wqT_sb, y2t)
            a2 = softmax("a2", l2)
            a2t = transpose4("a2t", a2)
            m2t = smm("m2t", wv_sb, a2t)             # M2^T
            m2pt = smm("m2pt", wphi_sb, m2t)         # (M2 @ Wphi)^T, fp32

            # ---------------- pass 2 ----------------
            for j in range(nch2):
                c0 = j * n2
                u2 = p2_in.tile([P, 2, n2], F32, name="u2")
                nc.sync.dma_start(u2, up_r[:, :, c0:c0 + n2])
                dn2 = p2_in.tile([P, 2, 2 * n2], F32, name="dn2")
                nc.sync.dma_start(dn2, dn_r[:, :, 2 * c0:2 * c0 + 2 * n2])
                d162 = p2_in.tile([P, 2, n2], BF16, name="d162")
                nc.sync.dma_start(d162, d16_dram[:, :, c0:c0 + n2])

                upo = p2_out.tile([P, 2, n2], F32, name="upo")
                dno = p2_out.tile([P, 2, 2 * n2], F32, name="dno")
                dnop = dno.rearrange("p m (n two) -> p m n two", two=2)
                dn2p = dn2.rearrange("p m (n two) -> p m n two", two=2)
                for m in range(2):
                    ms = slice(m * P, (m + 1) * P)
                    ps1 = work_psum.tile([P, n2], F32, name="ps1", tag="wps")
                    for kk in range(2):
                        nc.tensor.matmul(ps1, m1t16[:, kk, ms], d162[:, kk, :],
                                         start=(kk == 0), stop=(kk == 1))
                    nc.vector.tensor_add(upo[:, m, :], u2[:, m, :], ps1)

                    ps2 = work_psum.tile([P, n2], F32, name="ps2", tag="wps")
                    for kk in range(2):
                        nc.tensor.matmul(ps2, m2pt[:, kk, ms], u2[:, kk, :],
                                         start=(kk == 0), stop=(kk == 1))
                    nc.vector.tensor_add(dnop[:, m, :, 0], dn2p[:, m, :, 0], ps2)
                    nc.vector.tensor_add(dnop[:, m, :, 1], dn2p[:, m, :, 1], ps2)
                nc.sync.dma_start(upo_r[:, :, c0:c0 + n2], upo)
                nc.sync.dma_start(dno_r[:, :, 2 * c0:2 * c0 + 2 * n2], dno)

    nc.compile()
    return nc


def make_weight_inputs(vk_w, vk_b, q_w, q_b, psi1_w, psi1_b, psi2_w, psi2_b,
                       phi_w, phi_b):
    """Host-side packing of the derived weight tensors."""
    for name, b in [("vk_b", vk_b), ("q_b", q_b), ("psi1_b", psi1_b),
                    ("psi2_b", psi2_b), ("phi_b", phi_b)]:
        assert np.max(np.abs(np.asarray(b))) == 0.0, (
            f"{name} is nonzero; this kernel is specialized for zero biases")

    Wv = np.asarray(vk_w[:C, :], np.float32)
    Wk = np.asarray(vk_w[C:, :], np.float32)
    Wq = np.asarray(q_w, np.float32)
    Wphi = np.eye(C, dtype=np.float32) + np.asarray(phi_w, np.float32)
    W2f = np.einsum("ock,ci->oik", np.asarray(psi2_w, np.float32),
                    np.asarray(psi1_w, np.float32))
    w2f_packed = np.stack(
        [_pack_lhsT(np.ascontiguousarray(W2f[:, :, k].T)) for k in range(3)]
    ).transpose(1, 0, 2, 3, 4)  # [128, 3, 2, 2, 128]
    return {
        "w2f": np.ascontiguousarray(w2f_packed),
        "wkT": _pack_lhsT(np.ascontiguousarray(Wk.T)),
        "wqT": _pack_lhsT(np.ascontiguousarray(Wq.T)),
        "wphiT": _pack_lhsT(np.ascontiguousarray(Wphi.T)),
        "wphi": _pack_lhsT(Wphi),
        "wv": _pack_lhsT(Wv),
    }


_CACHED = {}


def _get_program(LUP, LDOWN, n1, n2, num_devices):
    key = (LUP, LDOWN, n1, n2, num_devices)
    if key not in _CACHED:
        _CACHED[key] = build_program(LUP, LDOWN, n1, n2, num_devices)
    return _CACHED[key]


def kernel(up_input, down_input, vk_w, vk_b, q_w, q_b,
           psi1_w, psi1_b, psi2_w, psi2_b, phi_w, phi_b):
    from concourse.bass_utils import run_bass_kernel_spmd

    up_input = np.asarray(up_input, np.float32)
    down_input = np.asarray(down_input, np.float32)
    B = up_input.shape[0]
    LUP = up_input.shape[2]
    LDOWN = down_input.shape[2]
    assert B == 8 and up_input.shape[1] == C

    nc = _get_program(LUP, LDOWN, 512, 512, B)
    weights = make_weight_inputs(vk_w, vk_b, q_w, q_b, psi1_w, psi1_b,
                                 psi2_w, psi2_b, phi_w, phi_b)
    in_maps = []
    for b in range(B):
        m = {"up": np.ascontiguousarray(up_input[b]),
             "down": np.ascontiguousarray(down_input[b])}
        m.update(weights)
        in_maps.append(m)

    res = run_bass_kernel_spmd(nc, in_maps, core_ids=list(range(B)))
    up_out = np.stack([res.results[b]["up_out"] for b in range(B)])
    down_out = np.stack([res.results[b]["down_out"] for b in range(B)])
    return up_out, down_out


if __name__ == "__main__":
    # scaled-down CoreSim validation (no hardware needed)
    from concourse.bass_interp import CoreSim

    LUP_S, LDOWN_S = 2048, 4096
    rng = np.random.default_rng(7)
    U = rng.standard_normal((C, LUP_S), np.float32)
    Dd = rng.standard_normal((C, LDOWN_S), np.float32)
    s = 0.02
    vk_w = (rng.standard_normal((2 * C, C)) * s).astype(np.float32)
    q_w = (rng.standard_normal((C, C)) * s).astype(np.float32)
    psi1_w = (rng.standard_normal((C, C)) * s).astype(np.float32)
    psi2_w = (rng.standard_normal((C, C, 3)) * s).astype(np.float32)
    phi_w = (rng.standard_normal((C, C)) * s).astype(np.float32)
    zeros = np.zeros(C, np.float32)

    # numpy golden (fp32)
    def golden(U, Dd):
        Wv, Wk, Wq = vk_w[:C], vk_w[C:], q_w
        Wphi = np.eye(C, dtype=np.float32) + phi_w
        W2f = np.einsum("ock,ci->oik", psi2_w, psi1_w)
        Dpad = np.concatenate([np.full((C, 1), -np.inf, np.float32), Dd,
                               np.full((C, 1), -np.inf, np.float32)], 1)
        mp = np.maximum(np.maximum(Dpad[:, 0:-2:2], Dpad[:, 1:-1:2]),
                        Dpad[:, 2::2])
        Dz = np.concatenate([np.zeros((C, 1), np.float32), Dd,
                             np.zeros((C, 1), np.float32)], 1)
        LUP_ = Dd.shape[1] // 2
        conv = sum(W2f[:, :, k] @ Dz[:, k:k + Dd.shape[1]:2][:, :LUP_]
                   for k in range(3))
        D = mp + conv
        Dsum = Dd[:, 0::2] + Dd[:, 1::2]
        G1 = U @ D.T
        H = Dsum @ U.T

        def sm(x):
            e = np.exp(x - x.max(-1, keepdims=True))
            return e / e.sum(-1, keepdims=True)

        A1 = sm((Wq @ G1 @ Wk.T) / SC)
        up_out = U + (A1 @ Wv) @ D
        DdU2D = H @ Wphi.T
        A2 = sm((Wq @ DdU2D @ Wk.T) / SC)
        Rt = (A2 @ Wv @ Wphi) @ U
        dn_out = Dd.copy()
        dn_out[:, 0::2] += Rt
        dn_out[:, 1::2] += Rt
        return up_out, dn_out

    g_up, g_dn = golden(U, Dd)

    nc = build_program(LUP_S, LDOWN_S, 512, 512, num_devices=1)
    weights = make_weight_inputs(vk_w, zeros * 0, q_w, zeros, psi1_w, zeros,
                                 psi2_w, zeros, phi_w, zeros)
    sim = CoreSim(nc, trace=False)
    sim.tensor("up")[:] = U
    sim.tensor("down")[:] = Dd
    for k, v in weights.items():
        sim.tensor(k)[:] = v
    sim.simulate(check_with_hw=False)
    out_up = np.array(sim.tensor("up_out"))
    out_dn = np.array(sim.tensor("down_out"))

    def rel(a, r):
        return np.abs(a - r).max() / np.abs(r).max()

    print("sim up   rel err:", rel(out_up, g_up))
    print("sim down rel err:", rel(out_dn, g_dn))
    assert rel(out_up, g_up) < 5e-3 and rel(out_dn, g_dn) < 5e-3
    print("SIM OK")


# revision 11
# speedup vs baseline: 2.1750x; 2.1750x over previous
"""Trainium2 Bass kernel for nn_CrossLayerAggregation.

Strategy (data-parallel over batch, one batch element per NeuronCore):

Math refactoring (validated against the reference to ~1e-7 in fp32):
  * psi2(psi1(x)) is folded into a single stride-2 conv3 with weights
    W2f[:,:,k] = psi2_w[:,:,k] @ psi1_w  (biases are zero in this problem).
  * nearest-upsample x2 duplicates columns, so every up2down quantity
    lives at LUP resolution:  up2down[:, 2j] = up2down[:, 2j+1] = P[:, j]
    with P = (I + phi_w) @ U.
  * The [C,C] attention logits contract over L, and the tiny projection
    matrices commute out of the big matmuls:
        logits_d2u * sqrt(C) = Wq @ (U @ D^T) @ Wk^T
        logits_u2d * sqrt(C) = Wq @ (Dsum @ U^T) @ (I+phi)^T @ Wk^T
    where D = maxpool3s2(Dd) + conv3f(Dd) and Dsum[:,j] = Dd[:,2j]+Dd[:,2j+1].
  * Outputs:  up_out   = U  + (A1 @ Wv) @ D
              down_out = Dd + dup((A2 @ Wv @ (I+phi)) @ U)

Device pipeline per core:
  pass 1: stream U and Dd chunks; build D (PE conv + DVE maxpool) and Dsum;
          cast to bf16; xbar-DMA-transpose chunks to [l, c] layout; accumulate
          G1^T = D U^T-style contractions in PSUM.  D (bf16) is staged to a
          DRAM scratch for pass 2.
  small stage: [256,256] logits chains, softmax, attention matrices (PE + DVE
          + ACT, all tiny).
  pass 2: stream U, Dd, D16 chunks; two small-K matmuls per chunk; residual
          adds; write outputs.
"""

import os
import sys
import math

sys.path.insert(0, "/opt/trn_rl_repo")

import numpy as np

import concourse.bass as bass
import concourse.tile as tile
from concourse import bacc, mybir
from concourse.masks import make_identity

F32 = mybir.dt.float32
F16 = mybir.dt.float16  # 16-bit staging dtype (fp16: more mantissa than bf16, data is O(10))
AF = mybir.ActivationFunctionType

C = 256
P = 128
SC = math.sqrt(C)


def _pack_lhsT(w):
    """[256,256] -> [128, 2, 2, 128]; tile (kk, m) = w[kk*128:(kk+1)*128, m*128:(m+1)*128].

    smm() with this packing computes  out = w.T @ rhs.
    """
    return np.ascontiguousarray(
        w.reshape(2, P, 2, P).transpose(1, 0, 2, 3)).astype(np.float16)


def build_program(LUP, LDOWN, n1, n2, num_devices=8, phase='all'):
    """Build the Bass program. n1: pass-1 chunk (l-cols), n2: pass-2 superchunk."""
    assert LDOWN == 2 * LUP
    nch1 = LUP // n1
    nch2 = LUP // n2
    nblk = n1 // P
    assert n1 == 512 and n2 % 1024 == 0

    nc = bacc.Bacc("TRN2", target_bir_lowering=False, debug=False,
                   num_devices=num_devices)

    up_d = nc.dram_tensor("up", [C, LUP], F32, kind="ExternalInput")
    dn_d = nc.dram_tensor("down", [C, LDOWN], F32, kind="ExternalInput")
    w2f_d = nc.dram_tensor("w2f", [P, 3, 2, 2, P], F16, kind="ExternalInput")
    wkT_d = nc.dram_tensor("wkT", [P, 2, 2, P], F16, kind="ExternalInput")
    wqT_d = nc.dram_tensor("wqT", [P, 2, 2, P], F16, kind="ExternalInput")
    wphiT_d = nc.dram_tensor("wphiT", [P, 2, 2, P], F16, kind="ExternalInput")
    wphi_d = nc.dram_tensor("wphi", [P, 2, 2, P], F16, kind="ExternalInput")
    wv_d = nc.dram_tensor("wv", [P, 2, 2, P], F16, kind="ExternalInput")
    upo_d = nc.dram_tensor("up_out", [C, LUP], F32, kind="ExternalOutput")
    dno_d = nc.dram_tensor("down_out", [C, LDOWN], F32, kind="ExternalOutput")

    up_r = up_d.ap().rearrange("(m p) l -> p m l", p=P)
    dn_r = dn_d.ap().rearrange("(m p) l -> p m l", p=P)
    upo_r = upo_d.ap().rearrange("(m p) l -> p m l", p=P)
    dno_r = dno_d.ap().rearrange("(m p) l -> p m l", p=P)

    with tile.TileContext(nc) as tc:
        with (
            tc.tile_pool(name="singles", bufs=1) as singles,
            tc.tile_pool(name="small", bufs=1) as small,
            tc.tile_pool(name="dram", bufs=1, space="DRAM") as dram,
            tc.tile_pool(name="p1_in", bufs=2) as p1_in,
            tc.tile_pool(name="p1_work", bufs=2) as p1_work,
            tc.tile_pool(name="p1_tr", bufs=2) as p1_tr,
            tc.tile_pool(name="work_psum", bufs=4, space="PSUM") as work_psum,
            tc.tile_pool(name="acc_pool", bufs=1, space="PSUM") as acc_pool,
            tc.tile_pool(name="p2_ld", bufs=2) as p2_ld,
            tc.tile_pool(name="p2_out", bufs=2) as p2_out,
        ):
            ident = singles.tile([P, P], F16, name="ident")
            make_identity(nc, ident)

            w2f_sb = singles.tile([P, 3, 2, 2, P], F16, name="w2f_sb")
            nc.sync.dma_start(w2f_sb, w2f_d.ap())
            wkT_sb = singles.tile([P, 2, 2, P], F16, name="wkT_sb")
            nc.sync.dma_start(wkT_sb, wkT_d.ap())
            wqT_sb = singles.tile([P, 2, 2, P], F16, name="wqT_sb")
            nc.sync.dma_start(wqT_sb, wqT_d.ap())
            wphiT_sb = singles.tile([P, 2, 2, P], F16, name="wphiT_sb")
            nc.sync.dma_start(wphiT_sb, wphiT_d.ap())
            wphi_sb = singles.tile([P, 2, 2, P], F16, name="wphi_sb")
            nc.sync.dma_start(wphi_sb, wphi_d.ap())
            wv_sb = singles.tile([P, 2, 2, P], F16, name="wv_sb")
            nc.sync.dma_start(wv_sb, wv_d.ap())

            d16_dram = dram.tile([P, 2, LUP], F16, name="d16_dram")

            # ---------------- pass 1 ----------------
            # one PSUM bank per accumulation group (start=True clears the
            # whole bank); the small stage reuses these banks via tags.
            g1_ps = [acc_pool.tile([P, C], F32, name=f"g1_ps{m}", tag=f"acc{m}")
                     for m in range(2)]
            ht_ps = [acc_pool.tile([P, C], F32, name=f"ht_ps{m}", tag=f"acc{2+m}")
                     for m in range(2)]

            for i in range(nch1 if phase in ('all', 'p1') else 0):
                c0 = i * n1
                # fp16 inputs via SWDGE cast-DMA (gpsimd)
                u16 = p1_in.tile([P, 2, n1], F16, name="u16")
                nc.gpsimd.dma_start(u16, up_r[:, :, c0:c0 + n1])
                # 2*n1 data cols + left halo col + 1 never-read pad col
                dn16 = p1_in.tile([P, 2, 2 * n1 + 2], F16, name="dn16")
                if i == 0:
                    nc.vector.memset(dn16[:, :, 0:1], 0.0)
                    nc.gpsimd.dma_start(dn16[:, :, 1:2 * n1 + 1],
                                        dn_r[:, :, 0:2 * n1])
                else:
                    nc.gpsimd.dma_start(dn16[:, :, 0:2 * n1 + 1],
                                        dn_r[:, :, 2 * c0 - 1:2 * c0 + 2 * n1])

                # maxpool3 stride2: out[t] = max(x[2t-1], x[2t], x[2t+1])
                dnp = dn16[:, :, 1:1 + 2 * n1].rearrange(
                    "p m (n two) -> p m n two", two=2)
                sA = dnp[:, :, :, 0]
                sB = dnp[:, :, :, 1]
                sCm = dn16[:, :, 0:2 * n1].rearrange(
                    "p m (n two) -> p m n two", two=2)[:, :, :, 0]
                t1 = p1_work.tile([P, 2, n1], F16, name="t1")
                nc.vector.tensor_max(t1, sA, sB)
                nc.vector.tensor_max(t1, t1, sCm)
                if i == 0:
                    # column 0 window is [-inf-pad, x0, x1]; the shared halo
                    # column holds conv's 0.0 — recompute without it.
                    nc.vector.tensor_max(t1[:, :, 0:1], dn16[:, :, 1:2],
                                         dn16[:, :, 2:3])

                # folded conv3 stride-2 (PE), D16 = maxpool + conv
                d16 = p1_work.tile([P, 2, n1], F16, name="d16")
                for m in range(2):
                    psc = work_psum.tile([P, n1], F32, name="psc", tag="wps")
                    for k in range(3):
                        for kk in range(2):
                            rhs = dn16[:, kk, k:k + 2 * n1].rearrange(
                                "p (n two) -> p n two", two=2)[:, :, 0]
                            nc.tensor.matmul(
                                psc, w2f_sb[:, k, kk, m, :], rhs,
                                start=(k == 0 and kk == 0),
                                stop=(k == 2 and kk == 1))
                    nc.vector.tensor_add(d16[:, m, :], t1[:, m, :], psc)
                nc.scalar.dma_start(d16_dram[:, :, c0:c0 + n1], d16)

                # Dsum in fp16
                s16 = p1_work.tile([P, 2, n1], F16, name="s16")
                nc.vector.tensor_add(s16, sA, sB)

                # one xbar transpose per tensor: [128, 2*n1] -> [128, 2*nblk, 128]
                # block b = m*nblk + lb  (m-major), so [l, c]-tiles are read with
                # a strided AP in the contraction below.
                ut = p1_tr.tile([P, 2 * nblk, P], F16, name="ut")
                dt_ = p1_tr.tile([P, 2 * nblk, P], F16, name="dt_")
                st = p1_tr.tile([P, 2 * nblk, P], F16, name="st")
                nc.scalar.dma_start(ut, u16.rearrange("p m l -> p (m l)"),
                                    transpose=True)
                nc.scalar.dma_start(dt_, d16.rearrange("p m l -> p (m l)"),
                                    transpose=True)
                nc.scalar.dma_start(st, s16.rearrange("p m l -> p (m l)"),
                                    transpose=True)

                ut_r = ut.rearrange("p (m b) j -> p b m j", m=2)
                for b in range(nblk):
                    st_mm = (i == 0 and b == 0)
                    sp_mm = (i == nch1 - 1 and b == nblk - 1)
                    rhs_u = ut_r[:, b, :, :]
                    for m in range(2):
                        nc.tensor.matmul(g1_ps[m], dt_[:, m * nblk + b, :],
                                         rhs_u, start=st_mm, stop=sp_mm,
                                         skip_group_check=True)
                    st_r = st.rearrange("p (m b) j -> p b m j", m=2)[:, b, :, :]
                    for m in range(2):
                        nc.tensor.matmul(ht_ps[m], ut[:, m * nblk + b, :],
                                         st_r, start=st_mm, stop=sp_mm,
                                         skip_group_check=True)

            # ---------------- small stage (all fp16 in/out, fp32 psum) -------
            def psum_to_sbuf(name, ps_tiles):
                out = small.tile([P, 2, C], F16, name=name)
                for m in range(2):
                    nc.scalar.activation(out[:, m, :], ps_tiles[m], AF.Copy)
                return out

            if phase in ('all', 'p1'):
                g1_sb = psum_to_sbuf("g1_sb", g1_ps)   # = G1^T  [d, c]
                ht_sb = psum_to_sbuf("ht_sb", ht_ps)   # = H^T   [d, c]

            def smm(name, w_sb, rhs):
                """out = W.T @ rhs for packed w_sb; rhs/out are [P, 2, C] fp16."""
                out = small.tile([P, 2, C], F16, name=name)
                for m in range(2):
                    ps = acc_pool.tile([P, C], F32, name=f"{name}_ps", tag="acc0")
                    for kk in range(2):
                        nc.tensor.matmul(ps, w_sb[:, kk, m, :], rhs[:, kk, :],
                                         start=(kk == 0), stop=(kk == 1))
                    nc.scalar.activation(out[:, m, :], ps, AF.Copy)
                return out

            def transpose4(name, src):
                out = small.tile([P, 2, C], F16, name=name)
                for mi in range(2):
                    for mj in range(2):
                        ps = acc_pool.tile([P, P], F16, name=f"{name}_tp",
                                           tag="acc1")
                        nc.tensor.transpose(
                            ps, src[:, mi, mj * P:(mj + 1) * P], ident)
                        nc.scalar.activation(
                            out[:, mj, mi * P:(mi + 1) * P], ps, AF.Copy)
                return out

            def softmax(name, src):
                out = small.tile([P, 2, C], F16, name=name)
                for m in range(2):
                    mx = small.tile([P, 1], F32, name=f"{name}_mx{m}")
                    nc.vector.reduce_max(out=mx, in_=src[:, m, :],
                                         axis=mybir.AxisListType.X)
                    nm = small.tile([P, 1], F32, name=f"{name}_nm{m}")
                    nc.vector.tensor_scalar_mul(nm, mx, -1.0 / SC)
                    e = small.tile([P, C], F32, name=f"{name}_e{m}")
                    nc.scalar.activation(e, src[:, m, :], AF.Exp,
                                         bias=nm, scale=1.0 / SC)
                    s = small.tile([P, 1], F32, name=f"{name}_s{m}")
                    nc.vector.reduce_sum(out=s, in_=e, axis=mybir.AxisListType.X)
                    r = small.tile([P, 1], F32, name=f"{name}_r{m}")
                    nc.vector.reciprocal(r, s)
                    nc.vector.tensor_scalar_mul(out[:, m, :], e, r)
                return out

            if phase == 'p2':
                m1t16 = small.tile([P, 2, C], F16, name="m1t16")
                nc.vector.memset(m1t16, 0.01)
                m2pt16 = small.tile([P, 2, C], F16, name="m2pt16")
                nc.vector.memset(m2pt16, 0.01)
            # up path attention
            y1 = smm("y1", wkT_sb, g1_sb)            # Wk @ G1^T
            y1t = transpose4("y1t", y1)              # G1 @ Wk^T
            l1 = smm("l1", wqT_sb, y1t)              # logits1 * SC  [c, d]
            a1 = softmax("a1", l1)
            a1t = transpose4("a1t", a1)
            m1t16 = smm("m1t16", wv_sb, a1t)         # M1^T = Wv^T @ A1^T

            # down path attention
            ddu2dt = smm("ddu2dt", wphiT_sb, ht_sb)  # (Dd @ U2D^T)^T
            y2 = smm("y2", wkT_sb, ddu2dt)
            y2t = transpose4("y2t", y2)
            l2 = smm("l2", wqT_sb, y2t)
            a2 = softmax("a2", l2)
            a2t = transpose4("a2t", a2)
            m2t = smm("m2t", wv_sb, a2t)             # M2^T
            m2pt16 = smm("m2pt16", wphi_sb, m2t)     # (M2 @ Wphi)^T

            # ---------------- pass 2 (superchunks of n2 cols) ----------------
            for j in range(nch2 if phase in ('all', 'p2') else 0):
                c0 = j * n2
                u2 = p2_ld.tile([P, 2, n2], F32, name="u2")
                nc.sync.dma_start(u2, up_r[:, :, c0:c0 + n2])
                dn2 = p2_ld.tile([P, 2, 2 * n2], F32, name="dn2")
                nc.sync.dma_start(dn2, dn_r[:, :, 2 * c0:2 * c0 + 2 * n2])
                d162 = p2_ld.tile([P, 2, n2], F16, name="d162")
                nc.scalar.dma_start(d162, d16_dram[:, :, c0:c0 + n2])
                u216 = p2_ld.tile([P, 2, n2], F16, name="u216")
                nc.scalar.activation(u216, u2, AF.Copy)

                upo = p2_out.tile([P, 2, n2], F32, name="upo")
                dno = p2_out.tile([P, 2, 2 * n2], F32, name="dno")
                dnop = dno.rearrange("p m (n two) -> p m n two", two=2)
                dn2p = dn2.rearrange("p m (n two) -> p m n two", two=2)
                for h in range(n2 // 512):
                    hs = slice(h * 512, (h + 1) * 512)
                    for m in range(2):
                        ms = slice(m * P, (m + 1) * P)
                        ps1 = work_psum.tile([P, 512], F32, name="ps1", tag="wps")
                        for kk in range(2):
                            nc.tensor.matmul(ps1, m1t16[:, kk, ms],
                                             d162[:, kk, hs],
                                             start=(kk == 0), stop=(kk == 1))
                        nc.vector.tensor_add(upo[:, m, hs], u2[:, m, hs], ps1)

                        ps2 = work_psum.tile([P, 512], F32, name="ps2", tag="wps")
                        for kk in range(2):
                            nc.tensor.matmul(ps2, m2pt16[:, kk, ms],
                                             u216[:, kk, hs],
                                             start=(kk == 0), stop=(kk == 1))
                        nc.vector.tensor_add(dnop[:, m, hs, 0],
                                             dn2p[:, m, hs, 0], ps2)
                        nc.vector.tensor_add(dnop[:, m, hs, 1],
                                             dn2p[:, m, hs, 1], ps2)
                nc.sync.dma_start(upo_r[:, :, c0:c0 + n2], upo)
                nc.sync.dma_start(dno_r[:, :, 2 * c0:2 * c0 + 2 * n2], dno)

    nc.compile()
    return nc


def make_weight_inputs(vk_w, vk_b, q_w, q_b, psi1_w, psi1_b, psi2_w, psi2_b,
                       phi_w, phi_b):
    """Host-side packing of the derived weight tensors."""
    for name, b in [("vk_b", vk_b), ("q_b", q_b), ("psi1_b", psi1_b),
                    ("psi2_b", psi2_b), ("phi_b", phi_b)]:
        assert np.max(np.abs(np.asarray(b))) == 0.0, (
            f"{name} is nonzero; this kernel is specialized for zero biases")

    Wv = np.asarray(vk_w[:C, :], np.float32)
    Wk = np.asarray(vk_w[C:, :], np.float32)
    Wq = np.asarray(q_w, np.float32)
    Wphi = np.eye(C, dtype=np.float32) + np.asarray(phi_w, np.float32)
    W2f = np.einsum("ock,ci->oik", np.asarray(psi2_w, np.float32),
                    np.asarray(psi1_w, np.float32))
    w2f_packed = np.stack(
        [_pack_lhsT(np.ascontiguousarray(W2f[:, :, k].T)) for k in range(3)]
    ).transpose(1, 0, 2, 3, 4)  # [128, 3, 2, 2, 128]
    return {
        "w2f": np.ascontiguousarray(w2f_packed),
        "wkT": _pack_lhsT(np.ascontiguousarray(Wk.T)),
        "wqT": _pack_lhsT(np.ascontiguousarray(Wq.T)),
        "wphiT": _pack_lhsT(np.ascontiguousarray(Wphi.T)),
        "wphi": _pack_lhsT(Wphi),
        "wv": _pack_lhsT(Wv),
    }


_CACHED = {}


def _get_program(LUP, LDOWN, n1, n2, num_devices):
    key = (LUP, LDOWN, n1, n2, num_devices)
    if key not in _CACHED:
        _CACHED[key] = build_program(LUP, LDOWN, n1, n2, num_devices)
    return _CACHED[key]


def kernel(up_input, down_input, vk_w, vk_b, q_w, q_b,
           psi1_w, psi1_b, psi2_w, psi2_b, phi_w, phi_b):
    from concourse.bass_utils import run_bass_kernel_spmd

    up_input = np.asarray(up_input, np.float32)
    down_input = np.asarray(down_input, np.float32)
    B = up_input.shape[0]
    LUP = up_input.shape[2]
    LDOWN = down_input.shape[2]
    assert B == 8 and up_input.shape[1] == C

    nc = _get_program(LUP, LDOWN, 512, 1024, B)
    weights = make_weight_inputs(vk_w, vk_b, q_w, q_b, psi1_w, psi1_b,
                                 psi2_w, psi2_b, phi_w, phi_b)
    in_maps = []
    for b in range(B):
        m = {"up": np.ascontiguousarray(up_input[b]),
             "down": np.ascontiguousarray(down_input[b])}
        m.update(weights)
        in_maps.append(m)

    res = run_bass_kernel_spmd(nc, in_maps, core_ids=list(range(B)))
    up_out = np.stack([res.results[b]["up_out"] for b in range(B)])
    down_out = np.stack([res.results[b]["down_out"] for b in range(B)])
    return up_out, down_out


if __name__ == "__main__":
    # scaled-down CoreSim validation (no hardware needed)
    from concourse.bass_interp import CoreSim

    LUP_S, LDOWN_S = 2048, 4096
    rng = np.random.default_rng(7)
    U = rng.standard_normal((C, LUP_S), np.float32)
    Dd = rng.standard_normal((C, LDOWN_S), np.float32)
    s = 0.02
    vk_w = (rng.standard_normal((2 * C, C)) * s).astype(np.float32)
    q_w = (rng.standard_normal((C, C)) * s).astype(np.float32)
    psi1_w = (rng.standard_normal((C, C)) * s).astype(np.float32)
    psi2_w = (rng.standard_normal((C, C, 3)) * s).astype(np.float32)
    phi_w = (rng.standard_normal((C, C)) * s).astype(np.float32)
    zeros = np.zeros(C, np.float32)

    # numpy golden (fp32)
    def golden(U, Dd):
        Wv, Wk, Wq = vk_w[:C], vk_w[C:], q_w
        Wphi = np.eye(C, dtype=np.float32) + phi_w
        W2f = np.einsum("ock,ci->oik", psi2_w, psi1_w)
        Dpad = np.concatenate([np.full((C, 1), -np.inf, np.float32), Dd,
                               np.full((C, 1), -np.inf, np.float32)], 1)
        mp = np.maximum(np.maximum(Dpad[:, 0:-2:2], Dpad[:, 1:-1:2]),
                        Dpad[:, 2::2])
        Dz = np.concatenate([np.zeros((C, 1), np.float32), Dd,
                             np.zeros((C, 1), np.float32)], 1)
        LUP_ = Dd.shape[1] // 2
        conv = sum(W2f[:, :, k] @ Dz[:, k:k + Dd.shape[1]:2][:, :LUP_]
                   for k in range(3))
        D = mp + conv
        Dsum = Dd[:, 0::2] + Dd[:, 1::2]
        G1 = U @ D.T
        H = Dsum @ U.T

        def sm(x):
            e = np.exp(x - x.max(-1, keepdims=True))
            return e / e.sum(-1, keepdims=True)

        A1 = sm((Wq @ G1 @ Wk.T) / SC)
        up_out = U + (A1 @ Wv) @ D
        DdU2D = H @ Wphi.T
        A2 = sm((Wq @ DdU2D @ Wk.T) / SC)
        Rt = (A2 @ Wv @ Wphi) @ U
        dn_out = Dd.copy()
        dn_out[:, 0::2] += Rt
        dn_out[:, 1::2] += Rt
        return up_out, dn_out

    g_up, g_dn = golden(U, Dd)

    nc = build_program(LUP_S, LDOWN_S, 512, 1024, num_devices=1)
    weights = make_weight_inputs(vk_w, zeros * 0, q_w, zeros, psi1_w, zeros,
                                 psi2_w, zeros, phi_w, zeros)
    sim = CoreSim(nc, trace=False)
    sim.tensor("up")[:] = U
    sim.tensor("down")[:] = Dd
    for k, v in weights.items():
        sim.tensor(k)[:] = v
    sim.simulate(check_with_hw=False)
    out_up = np.array(sim.tensor("up_out"))
    out_dn = np.array(sim.tensor("down_out"))

    def rel(a, r):
        return np.abs(a - r).max() / np.abs(r).max()

    print("sim up   rel err:", rel(out_up, g_up))
    print("sim down rel err:", rel(out_dn, g_dn))
    assert rel(out_up, g_up) < 5e-3 and rel(out_dn, g_dn) < 5e-3
    print("SIM OK")
